# revision 6
# baseline (speedup 1.0000x reference)
"""Trainium2 Bass kernel for DeformationTrackerBiFlowModel — G=9 split-matmul.

Reference math (per batch element b, per step t):
    x_t   = [prev_out (2), fin_t (3)]            (5,)
    h_t   = tanh(x_t @ W_rnn + b_rnn)            (12,)   (U_rnn is inert)
    out_t = [cp0 (2), h_t (12)] @ W_out + b_out  (2,)
    prev_out_{t+1} = out_t;  prev_out_0 = cp0

Unrolled one step:
    pre_t = h_{t-1}@Wh + fin_t@W1f + cp0@E + r      Wh=Wo2@W1p, E=Wo1@W1p,
    out_{t-1} = h_{t-1}@Wo2 + cp0@Wo1 + b_out       r=b_out@W1p+b_rnn
    h_t = tanh(pre_t)

G=9 trajectories are packed block-diagonally. Splitting the contraction into
two PE matmuls per round lifts the K<=128 limit on G:
    fin-mm (K = 3G fin + 1 ones + 2G cp0 = 46, start=True):  fin/const terms
      of BOTH pre rows [0:108] and out rows [108:126] (b_out, Wo1 live here).
    h-mm   (K = 12G = 108, accumulate): h@Wh into pre, h@Wo2 into out.
M = 14G = 126 <= 128; one psum bank holds [126, 456] f32.

The out rows ride the tanh: their weight columns are pre-scaled by S=1/32 so
psum[108:126] = S*out with |S*out| << 1, and the single per-round ACT tanh
covers all 126 partitions (engine time only counts the free dim, so the 18
extra rows are free). tanh(S*out) lands in rows 108:126 of the same rhs tile
the h rows go to — no DVE/Pool copy, no separate staging; the host inverts
with arctanh(y)/S during unstaging. Out rows are DMA'd 4 steps per transfer
out of the 8-deep rhs block ring (blocks are step-contiguous under the
(round+1)%8 rotation with a phase-3 grouping).

Batch 65536 over 8 cores; per core G*C*COLS = 9*2*456 = 8208 (8192 + pad 16).
Round 0 is fin-mm only (w0 variant: ones->b_rnn, cp0->W1p); round T emits
only out_{T-1} (stale fin rows have zero out-columns).
"""

import os
from contextlib import ExitStack

import numpy as np

import concourse.mybir as mybir
import concourse.tile as tile
from concourse import bacc
from concourse.bass_utils import run_bass_kernel_spmd

B, T = 65536, 100
D_CP, D_FIN, HID = 2, 3, 12
NCORES = 8
BC = B // NCORES              # 8192 per core
G = 9                         # trajectories packed per matmul (block-diag)
C = 2                         # independent column chains
COLS = 456                    # batch columns per chain
BP = G * C * COLS             # 8208 padded batch per core
NH = HID * G                  # 108: h rows (rhs of h-mm) / pre rows (psum)
NFIN = D_FIN * G              # 27 fin rows
NCONST = 1 + D_CP * G         # 19: ones + cp0 rows
KF = NFIN + NCONST            # 46: fin-mm contraction
MOUT = D_CP * G               # 18 out rows
MTOT = NH + MOUT              # 126 psum partitions
NBLK = 8                      # rhs h/out blocks
NBLK_F = 16                   # fin rhs blocks (deep ring so fin DMA runs 12 rounds ahead)
OSCALE = 1.0 / 32.0           # out-column weight scale so tanh(S*out)=S*out

F32 = mybir.dt.float32

_MM_CHOICES = {"bf16": mybir.dt.bfloat16, "f32r": mybir.dt.float32r, "f32": F32}
MM_DTYPE = _MM_CHOICES[os.environ.get("DTB_MM", "bf16")]
MM_NP = mybir.dt.np(MM_DTYPE)

LAST_RESULTS = None  # test.py introspects profiling info from here


def out_dma_groups(t_steps=T):
    """(emit_after_round, first_block, nblocks, first_step) covering all steps.

    Block (u+1)%NBLK holds step u-1; groups are chosen so src blocks are
    contiguous: head {rounds 1..3 -> blocks 2..4 -> steps 0..2}, then quads
    after round 4k+3 {blocks 4k+4.. -> steps 4k-1..4k+2}, tail {round
    t_steps -> step t_steps-1}.
    """
    groups = [(2, 2, 2, 0)]
    k = 0
    while 4 * k + 5 <= t_steps - 3:
        groups.append((4 * k + 6, (4 * k + 4) % NBLK, 4, 4 * k + 2))
        k += 1
    s0 = 4 * k + 2
    groups.append((t_steps, (s0 + 2) % NBLK, t_steps - s0, s0))
    return groups


def build_program(t_steps=T, g=G, c=C, cols=COLS, mm_dtype=None):
    if mm_dtype is None:
        mm_dtype = MM_DTYPE
    XDT = mm_dtype
    nh, nfin, nconst = HID * g, D_FIN * g, 1 + D_CP * g
    kf = nfin + nconst
    mout = D_CP * g
    mtot = nh + mout
    nc = bacc.Bacc(target_bir_lowering=False)

    fin = nc.dram_tensor("fin", [t_steps, c, nfin, cols], XDT, kind="ExternalInput")
    xc = nc.dram_tensor("xc", [c, nconst, NBLK_F * cols], XDT, kind="ExternalInput")
    wf = nc.dram_tensor("wf", [kf, mtot], XDT, kind="ExternalInput")
    w0f = nc.dram_tensor("w0f", [kf, mtot], XDT, kind="ExternalInput")
    wh = nc.dram_tensor("wh", [nh, mtot], XDT, kind="ExternalInput")
    out = nc.dram_tensor("out", [t_steps, c, mout, cols], XDT, kind="ExternalOutput")

    tanh = mybir.ActivationFunctionType.Tanh
    dma_groups = {}
    for rnd, blk0, nb, step0 in out_dma_groups(t_steps):
        dma_groups.setdefault(rnd, []).append((blk0, nb, step0))

    def quad_src(apn):
        return apn.rearrange("t r c -> r t c")

    with tile.TileContext(nc) as tc, ExitStack() as ctx:
        const = ctx.enter_context(tc.tile_pool(name="const", bufs=1))
        xpool = ctx.enter_context(tc.tile_pool(name="xpool", bufs=1))
        psum = ctx.enter_context(tc.tile_pool(name="psum", bufs=4, space="PSUM"))

        wfs = const.tile([kf, mtot], XDT, name="wfs")
        nc.sync.dma_start(out=wfs, in_=wf[:, :])
        w0fs = const.tile([kf, mtot], XDT, name="w0fs")
        nc.sync.dma_start(out=w0fs, in_=w0f[:, :])
        whs = const.tile([nh, mtot], XDT, name="whs")
        nc.sync.dma_start(out=whs, in_=wh[:, :])

        # Per-chain rhs tiles. fin tile rows: [fin (DMA 4 blocks/transfer) |
        # ones+cp0 (DMA once, host-tiled x8)]. ht rows: [h | tanh(S*out)],
        # written only by ACT; matmuls read rows 0:nh, out-DMAs rows nh:mtot.
        ftiles, htiles = [], []
        for ch in range(c):
            ft = xpool.tile([kf, NBLK_F * cols], XDT, tag=f"f{ch}", name=f"f_{ch}")
            nc.sync.dma_start(out=ft[nfin:, :], in_=xc[ch])
            for q in range(3):  # fin rounds 0..11 up front
                nc.sync.dma_start(
                    out=ft[
                        0:nfin, 4 * q * cols : 4 * (q + 1) * cols
                    ].rearrange("r (t c) -> r t c", t=4),
                    in_=quad_src(fin[4 * q : 4 * (q + 1), ch]),
                )
            ftiles.append(ft)
            ht = xpool.tile([mtot, NBLK * cols], XDT, tag=f"h{ch}", name=f"h_{ch}")
            htiles.append(ht)

        # psum tiles for rounds t..t+3 are alive together (bufs=4): fin-mms
        # are emitted in 4-round bursts so the PE sees same-stationary runs
        # of 8 (the LDWEIGHTS fast path + full p-state need runs, not pairs).
        pss = [[None] * c for _ in range(4)]

        def fin_mm(t):
            # fin-mm for round t (psum tile allocated here, start=True).
            for ch in range(c):
                ps = psum.tile([mtot, cols], F32, tag=f"p{ch}", name=f"p_{ch}_{t}")
                pss[t % 4][ch] = ps
                blk = t % NBLK_F
                nc.tensor.matmul(
                    ps, w0fs if t == 0 else wfs,
                    ftiles[ch][:, blk * cols : (blk + 1) * cols],
                    start=True, stop=(t == 0),
                )

        fin_mm(0)
        for t in range(t_steps + 1):
            for ch in range(c):
                if t > 0:
                    blk = t % NBLK
                    nc.tensor.matmul(
                        pss[t % 4][ch], whs,
                        htiles[ch][0:nh, blk * cols : (blk + 1) * cols],
                        start=False, stop=True,
                    )
            for ch in range(c):
                nb = (t + 1) % NBLK
                nc.scalar.activation(
                    htiles[ch][:, nb * cols : (nb + 1) * cols],
                    pss[t % 4][ch], tanh,
                )
            # Prefetch fin 4 steps per DMA, 12 rounds ahead (before the
            # burst so fin-mm(r) is emitted after the DMA that feeds it).
            s0 = t + 12
            if t % 4 == 0 and 12 <= s0 < t_steps:
                bs = s0 % NBLK_F
                for ch in range(c):
                    nc.sync.dma_start(
                        out=ftiles[ch][
                            0:nfin, bs * cols : (bs + 4) * cols
                        ].rearrange("r (t c) -> r t c", t=4),
                        in_=quad_src(fin[s0 : s0 + 4, ch]),
                    )
            if t % 4 == 0:  # burst: PE chews this while waiting on tanh_t
                for r in range(t + 1, min(t + 4, t_steps) + 1):
                    fin_mm(r)
            for blk0, nb_, step0 in dma_groups.get(t, ()):
                for ch in range(c):
                    nc.gpsimd.dma_start(
                        out=quad_src(out[step0 : step0 + nb_, ch]),
                        in_=htiles[ch][
                            nh:mtot, blk0 * cols : (blk0 + nb_) * cols
                        ].rearrange("r (t c) -> r t c", t=nb_),
                    )
    nc.compile()
    return nc


def build_packed_weights(W_rnn, W_out, b_rnn, b_out, g=G):
    W_rnn = np.asarray(W_rnn, np.float32)
    W_out = np.asarray(W_out, np.float32)
    b_rnn = np.asarray(b_rnn, np.float32)
    b_out = np.asarray(b_out, np.float32)
    W1p, W1f = W_rnn[:D_CP], W_rnn[D_CP:]
    Wo1, Wo2 = W_out[:D_CP], W_out[D_CP:]
    nh, nfin = HID * g, D_FIN * g
    kf = nfin + 1 + D_CP * g
    mtot = nh + D_CP * g
    ones_row = nfin
    cp0_base = nfin + 1
    S = np.float32(OSCALE)

    Wh = Wo2 @ W1p                     # (12, 12) h contribution to next pre
    E = Wo1 @ W1p                      # (2, 12) cp0 contribution to pre
    r = b_rnn + b_out @ W1p            # (12,) ones-row weight (steady state)

    wf_ = np.zeros((kf, mtot), np.float32)
    w0f_ = np.zeros((kf, mtot), np.float32)
    wh_ = np.zeros((nh, mtot), np.float32)
    for i in range(g):
        hsl = slice(HID * i, HID * (i + 1))
        osl = slice(nh + D_CP * i, nh + D_CP * (i + 1))
        wh_[hsl, hsl] = Wh
        wh_[hsl, osl] = Wo2 * S
        fsl = slice(D_FIN * i, D_FIN * (i + 1))
        wf_[fsl, hsl] = W1f
        w0f_[fsl, hsl] = W1f
        wf_[ones_row, hsl] = r
        w0f_[ones_row, hsl] = b_rnn
        wf_[ones_row, osl] = b_out * S
        csl = slice(cp0_base + D_CP * i, cp0_base + D_CP * (i + 1))
        wf_[csl, hsl] = E
        wf_[csl, osl] = Wo1 * S
        w0f_[csl, hsl] = W1p
    return wf_, w0f_, wh_


def stage_inputs(cp0, fin, g=G, c=C, cols=COLS, t_steps=T):
    """Batch-major -> feature-major device layouts (b = ch*(g*cols)+gi*cols+j)."""
    bp = g * c * cols
    bc = cp0.shape[0]
    fin_p = np.zeros((bp, t_steps, D_FIN), np.float32)
    fin_p[:bc] = fin
    cp0_p = np.zeros((bp, D_CP), np.float32)
    cp0_p[:bc] = cp0
    fin_d = np.ascontiguousarray(
        fin_p.reshape(c, g, cols, t_steps, D_FIN).transpose(3, 0, 1, 4, 2)
    ).reshape(t_steps, c, D_FIN * g, cols)
    xc_d = np.ones((c, 1 + D_CP * g, cols), np.float32)
    xc_d[:, 1:, :] = cp0_p.reshape(c, g, cols, D_CP).transpose(0, 1, 3, 2).reshape(
        c, D_CP * g, cols
    )
    xc_d = np.tile(xc_d, (1, 1, NBLK_F))
    return fin_d, xc_d


def unstage_output(out_d, bc, g=G, c=C, cols=COLS, t_steps=T):
    bp = g * c * cols
    o = out_d.reshape(t_steps, c, g, D_CP, cols).transpose(1, 2, 4, 0, 3)
    o = np.arctanh(np.ascontiguousarray(o).reshape(bp, t_steps, D_CP)[:bc])
    return o * np.float32(1.0 / OSCALE)


def kernel(control_point_input, finger_input, W_rnn, U_rnn, b_rnn, W_out, b_out):
    global LAST_RESULTS
    cp = np.asarray(control_point_input, np.float32)
    fin = np.asarray(finger_input, np.float32)

    cp0 = cp[:, 0, :]
    wf_, w0f_, wh_ = build_packed_weights(W_rnn, W_out, b_rnn, b_out)
    wf_, w0f_, wh_ = (x.astype(MM_NP) for x in (wf_, w0f_, wh_))

    nc = build_program()
    in_maps = []
    for m in range(NCORES):
        sl = slice(m * BC, (m + 1) * BC)
        fin_d, xc_d = stage_inputs(cp0[sl], fin[sl])
        in_maps.append(
            {"fin": fin_d.astype(MM_NP, copy=False),
             "xc": xc_d.astype(MM_NP, copy=False),
             "wf": wf_, "w0f": w0f_, "wh": wh_}
        )

    trace = bool(os.environ.get("DTB_TRACE"))
    res = run_bass_kernel_spmd(
        nc, in_maps, core_ids=list(range(NCORES)), trace=trace
    )
    LAST_RESULTS = res

    outs = [
        unstage_output(np.asarray(res.results[m]["out"], np.float32), BC)
        for m in range(NCORES)
    ]
    return np.concatenate(outs, axis=0)
